# revision 40
# baseline (speedup 1.0000x reference)
"""Trainium2 Bass kernel for nn_AutoEncoder_48799418417535.

VAE with 4 encoder + 6 decoder post-norm transformer layers (BigBird-style
block-sparse additive attention mask, applied here multiplicatively after exp),
data-parallel over batch across 8 NeuronCores (8 batches/core), params
replicated.

Device layout: activations live transposed [d(partitions), tok(free)] with
tok = b*256 + s (2048 tokens/core), so every projection runs with the weight
as the stationary operand and 512-column moving operands (full-rate float32r
matmuls). V is kept in natural layout [tok, dv] with a fused ones-column so
the attention softmax denominators fall out of the attnV matmul for free; the
normalization divide is deferred and done in bulk per (d-tile, batch).
LayerNorm reductions (over d = partitions) use ones-matmuls that replicate the
stats across all 128 partitions so the normalization runs full-width.
Weights are host-packed to [128, N] so each matrix is a single DMA.
"""

import sys

sys.path.insert(0, "/opt/trn_rl_repo")

import numpy as np

# ---- model dims (hardcoded from the problem spec) ----
NCORES = 8
BFULL, BC = 64, 8          # total batch, batch per core
S, TOK = 256, 8 * 256      # seq len, tokens per core
D, H, DH, FF, LAT, BLK = 512, 8, 64, 2048, 32, 16
NE, ND = 4, 6
NL = NE + ND

USE_F32R = True  # float32r matmuls: 4x faster than fp32, TF32-ish precision
import os
KDBG = bool(int(os.environ.get("KDBG", "0")))


# ---------------------------------------------------------------- host prep --
def _mask01T_packed():
    # replicate reference._build_mask() with numpy, as multiplicative 0/1,
    # transposed to [k, q], packed to [128, 2*256] (col kc*256+q, row k%128)
    rng = np.random.default_rng(0)
    m = np.zeros((32, 32), bool)
    m[:, :4] = True
    m[:4, :] = True
    half = 11 // 2
    for i in range(32):
        m[i, max(0, i - half):min(32, i + half + 1)] = True
        m[i, rng.choice(32, 6, replace=False)] = True
    tok = np.kron(m, np.ones((8, 8), bool))          # [q, k]
    mT = tok.T.astype(np.float32)                    # [k, q]
    return np.ascontiguousarray(
        mT.reshape(2, 128, 256).transpose(1, 0, 2).reshape(128, 512))


def _f(a):
    return np.ascontiguousarray(np.asarray(a, dtype=np.float32))


def _pack_cols(v, n):
    # [n*128] vector -> [128, n] (col j holds v[j*128 : (j+1)*128])
    return _f(v).reshape(n, 128).T


def _pack_mat(w):
    # [K, N] (K = 128*k) -> [128, k*N]: cols kt*N+j = w[kt*128+p, j]
    w = _f(w)
    k = w.shape[0] // 128
    return np.ascontiguousarray(
        w.reshape(k, 128, w.shape[1]).transpose(1, 0, 2).reshape(128, -1))


def _pos_packed(pos):
    # pos [256, 512] -> [128, 4*256]: cols dt*256+s = pos[s, dt*128+p]
    return np.ascontiguousarray(
        _f(pos).T.reshape(4, 128, 256).transpose(1, 0, 2).reshape(128, 1024))


def _host_prep(x, eps, p):
    """Build the full input map (common tensors + per-core xT17/epsT)."""
    common = {}
    m01 = _mask01T_packed()
    common["maskT8"] = np.where(m01 > 0, 0.0, -1e6).astype(np.float32)
    common["eye128"] = np.eye(128, dtype=np.float32)
    common["in_pos_pk"] = _pos_packed(p["in_pos"])
    common["out_pos_pk"] = _pos_packed(p["out_pos"])
    common["data_w17"] = np.vstack([_f(p["data_w"]), _f(p["data_b"])[None]])
    common["pm_w_pk"] = _pack_mat(p["pm_w"])      # [128, 128]
    common["pl_w_pk"] = _pack_mat(p["pl_w"])      # [128, 128]
    common["exp_w"] = _f(p["exp_w"])              # [32, 512]
    # conv_w [512(o), 2(i), 3(t)] -> [6=(i*3+t), 512]
    common["w6"] = np.ascontiguousarray(_f(p["conv_w"]).transpose(1, 2, 0).reshape(6, 512))
    common["seq_wT_pk"] = _pack_mat(_f(p["seq_w"]).T)  # [128, 64]
    gvp = np.zeros((128, 12), np.float32)
    gvp[0:32, 0] = _f(p["pm_b"])
    gvp[0:32, 1] = _f(p["pl_b"])
    gvp[:, 2:6] = _pack_cols(p["exp_b"], 4)
    gvp[:, 6:10] = _pack_cols(p["conv_b"], 4)
    gvp[0:16, 10] = _f(p["seq_b"])
    common["gvp"] = gvp
    common["ones128"] = np.ones((128, 128), np.float32)
    common["ones_v"] = np.ones((128, 32), np.float32)
    common["zeros16"] = np.zeros((1, 16), np.float32)

    for li in range(NL):
        pf = ("e%d_" % li) if li < NE else ("d%d_" % (li - NE))
        common[f"L{li}_Wq"] = _pack_mat(p[pf + "Wq"])   # [128, 2048]
        common[f"L{li}_Wk"] = _pack_mat(p[pf + "Wk"])
        common[f"L{li}_Wv"] = _pack_mat(p[pf + "Wv"])
        common[f"L{li}_Wo"] = _pack_mat(p[pf + "Wo"])
        common[f"L{li}_W1"] = _pack_mat(p[pf + "W1"])   # [128, 8192]
        common[f"L{li}_W2"] = _pack_mat(p[pf + "W2"])   # [128, 8192]
        vp = np.zeros((128, 44), np.float32)
        vp[:, 0:4] = _pack_cols(p[pf + "bq"], 4)
        # bv folded through Wo into the output bias (attn rows sum to 1)
        bo_eff = _f(p[pf + "bo"]) + _f(p[pf + "bv"]) @ _f(p[pf + "Wo"])
        vp[:, 4:8] = _pack_cols(bo_eff, 4)
        vp[:, 8:12] = _pack_cols(p[pf + "g1"], 4)
        vp[:, 12:16] = _pack_cols(p[pf + "c1"], 4)
        vp[:, 16:20] = _pack_cols(p[pf + "g2"], 4)
        vp[:, 20:24] = _pack_cols(p[pf + "c2"], 4)
        vp[:, 24:28] = _pack_cols(p[pf + "f2"], 4)
        vp[:, 28:44] = _pack_cols(p[pf + "f1"], 16)
        common[f"L{li}_vp"] = vp

    x = _f(x)      # [64, 255, 16]
    eps = _f(eps)  # [64, 32]
    in_maps = []
    for c in range(NCORES):
        xc = x[c * BC:(c + 1) * BC]                    # [8, 255, 16]
        xT17 = np.zeros((17, TOK), np.float32)
        xr = xT17[0:16].reshape(16, BC, S)
        xr[:, :, 1:] = xc.transpose(2, 0, 1)
        xT17[16].reshape(BC, S)[:, 1:] = 1.0           # bias-indicator row
        m = dict(common)
        m["xT17"] = xT17
        m["epsT"] = np.ascontiguousarray(eps[c * BC:(c + 1) * BC].T)  # [32, 8]
        in_maps.append(m)
    return in_maps


# ------------------------------------------------------------ device program --
def _build_program():
    import concourse.bass as bass
    import concourse.bacc as bacc
    import concourse.tile as tile
    from concourse import mybir
    from contextlib import ExitStack

    F32 = mybir.dt.float32
    F32R = mybir.dt.float32r
    AF = mybir.ActivationFunctionType
    ALU = mybir.AluOpType

    def R(ap):
        return ap.bitcast(F32R) if USE_F32R else ap

    W_ = R  # rounding write view (producers of matmul inputs)

    nc = bacc.Bacc("TRN2", target_bir_lowering=False)

    din = {}

    def dparam(name, shape, dt=None):
        din[name] = nc.declare_dram_parameter(
            name, list(shape), F32R if dt is None else dt, isOutput=False)
        return din[name]

    dparam("xT17", (17, TOK))
    dparam("epsT", (LAT, BC), F32)
    dparam("maskT8", (128, 512))
    dparam("eye128", (128, 128))
    dparam("in_pos_pk", (128, 1024), F32)
    dparam("out_pos_pk", (128, 1024), F32)
    dparam("data_w17", (17, D))
    dparam("pm_w_pk", (128, 128))
    dparam("pl_w_pk", (128, 128))
    dparam("exp_w", (LAT, D))
    dparam("w6", (6, D))
    dparam("seq_wT_pk", (128, 64))
    dparam("gvp", (128, 12), F32)
    dparam("ones128", (128, 128))
    dparam("ones_v", (128, 32))
    dparam("zeros16", (1, 16))
    for li in range(NL):
        dparam(f"L{li}_Wq", (128, 2048))
        dparam(f"L{li}_Wk", (128, 2048))
        dparam(f"L{li}_Wv", (128, 2048))
        dparam(f"L{li}_Wo", (128, 2048))
        dparam(f"L{li}_W1", (128, 8192))
        dparam(f"L{li}_W2", (128, 8192))
        dparam(f"L{li}_vp", (128, 44), F32)
    outT_d = nc.declare_dram_parameter("outT", [BLK, TOK], F32, isOutput=True)
    mean_d = nc.declare_dram_parameter("meanT", [LAT, BC], F32, isOutput=True)
    lv_d = nc.declare_dram_parameter("logvarT", [LAT, BC], F32, isOutput=True)
    E_scr = nc.dram_tensor("E_scr", [D, BC], F32R)

    dbg_outs = {}

    def _dbg_decl(name, ncols):
        for t in range(4):
            dbg_outs[f"{name}_{t}"] = nc.declare_dram_parameter(
                f"dbg_{name}_{t}", [128, ncols], F32, isOutput=True)

    def tcol(ch):
        return slice(ch * 512, ch * 512 + 512)

    def bcol(b):
        return slice(b * 256, b * 256 + 256)

    with ExitStack() as ctx:
        tc = ctx.enter_context(tile.TileContext(nc))
        big = ctx.enter_context(tc.tile_pool(name="big", bufs=1))
        wpool = ctx.enter_context(tc.tile_pool(name="wpool", bufs=4))
        vppool = ctx.enter_context(tc.tile_pool(name="vppool", bufs=2))
        statsp = ctx.enter_context(tc.tile_pool(name="stats", bufs=1))
        probsp = ctx.enter_context(tc.tile_pool(name="probs", bufs=3))
        frp = ctx.enter_context(tc.tile_pool(name="frp", bufs=3))
        smallp = ctx.enter_context(tc.tile_pool(name="smallp", bufs=1))
        initp = ctx.enter_context(tc.tile_pool(name="initp", bufs=1))
        pp = ctx.enter_context(tc.tile_pool(name="pp", bufs=4, space="PSUM"))
        ffpp = ctx.enter_context(tc.tile_pool(name="ffpp", bufs=1, space="PSUM"))

        # persistent activation buffers, each [512, 2080] as 4 tiles
        # roles: B0=h, B1=Q/A/scratch, B2=K/ln1, B3=V/t/u
        BUF = [[big.tile([128, 2080], F32, tag=f"B{i}_{t}", name=f"B{i}_{t}")
                for t in range(4)] for i in range(4)]
        Hb, QA, Kb, VU = BUF

        ones = big.tile([128, 128], F32, tag="ones")
        nc.sync.dma_start(out=W_(ones), in_=din["ones128"][:])
        epsc = big.tile([128, 1], F32, tag="epsc")
        nc.vector.memset(epsc, 1e-5)
        mask_sb = big.tile([128, 512], F32, tag="mask")
        nc.sync.dma_start(out=W_(mask_sb), in_=din["maskT8"][:])
        eye_sb = big.tile([128, 128], F32, tag="eye")
        nc.sync.dma_start(out=W_(eye_sb), in_=din["eye128"][:])
        gvp = big.tile([128, 12], F32, tag="gvp")
        nc.sync.dma_start(out=gvp, in_=din["gvp"][:])

        def posrep(pos_t, dt):
            # pos block dt [128,256] broadcast x2 along free -> [128, 2, 256]
            src = pos_t[:, dt * 256:(dt + 1) * 256]
            return bass.AP(tensor=src.tensor, offset=src.offset,
                           ap=[src.ap[0], [0, 2], src.ap[1]])

        def dbg(name, tiles, ncols=2048):
            if not KDBG:
                return
            _dbg_decl(name, ncols)
            for t in range(4):
                nc.sync.dma_start(out=dbg_outs[f"{name}_{t}"][:],
                                  in_=tiles[t][:, 0:ncols])

        def load_pk(drh, name):
            # one [128, n*512] packed weight DMA; slice kt/dt chunks later
            w = wpool.tile([128, drh.shape[1]], F32, tag="w", name=name)
            nc.sync.dma_start(out=W_(w), in_=drh[:])
            return w

        # ---------------- encoder init: h0 = concat(cls, x@W+b) + in_pos ----
        pos_t = wpool.tile([128, 1024], F32, tag="w", name="pos_t")
        nc.sync.dma_start(out=pos_t, in_=din["in_pos_pk"][:])
        xt = initp.tile([17, TOK], F32, tag="init")
        nc.sync.dma_start(out=W_(xt), in_=din["xT17"][:])
        dwt = wpool.tile([17, 512], F32, tag="w", name="dwt")
        nc.sync.dma_start(out=W_(dwt), in_=din["data_w17"][:])
        for dt in range(4):
            for ch in range(4):
                ps = pp.tile([128, 512], F32, tag="mm")
                nc.tensor.matmul(ps, R(dwt[:, dt * 128:(dt + 1) * 128]),
                                 R(xt[:, tcol(ch)]), start=True, stop=True)
                nc.vector.tensor_add(W_(Hb[dt][:, tcol(ch)]), ps, posrep(pos_t, dt))
        dbg("h0", Hb)

        # ---------------- transformer layer ----------------
        def layernorm_ch(X, SCR, OUT, gcol, ccol, ch, vp):
            for kt in range(4):
                nc.gpsimd.tensor_mul(W_(SCR[kt][:, tcol(ch)]),
                                     X[kt][:, tcol(ch)], X[kt][:, tcol(ch)])
            psu = ffpp.tile([128, 512], F32, tag=f"fa{(ch % 2) * 2}", name="psu")
            for kt in range(4):
                nc.tensor.matmul(psu, R(ones), R(X[kt][:, tcol(ch)]),
                                 start=kt == 0, stop=kt == 3)
            pss = ffpp.tile([128, 512], F32, tag=f"fa{(ch % 2) * 2 + 1}", name="pss")
            for kt in range(4):
                nc.tensor.matmul(pss, R(ones), R(SCR[kt][:, tcol(ch)]),
                                 start=kt == 0, stop=kt == 3)
            mu = statsp.tile([128, 512], F32, tag="mu", bufs=2)
            nc.scalar.activation(mu, psu, AF.Copy, scale=1.0 / 512)
            mu2 = statsp.tile([128, 512], F32, tag="tmp1")
            nc.scalar.activation(mu2, mu, AF.Square)
            var = statsp.tile([128, 512], F32, tag="tmp2")
            nc.vector.scalar_tensor_tensor(
                out=var, in0=pss, scalar=1.0 / 512, in1=mu2,
                op0=ALU.mult, op1=ALU.subtract)
            sd = statsp.tile([128, 512], F32, tag="tmp1")
            nc.scalar.activation(sd, var, AF.Sqrt, bias=epsc[:, 0:1])
            rstd = statsp.tile([128, 512], F32, tag="rstd")
            nc.vector.reciprocal_approx_fast(rstd, sd)
            for kt in range(4):
                nc.vector.tensor_sub(W_(OUT[kt][:, tcol(ch)]),
                                     X[kt][:, tcol(ch)], mu)
                nc.vector.tensor_mul(W_(OUT[kt][:, tcol(ch)]),
                                     OUT[kt][:, tcol(ch)], rstd)
                nc.scalar.activation(W_(OUT[kt][:, tcol(ch)]),
                                     OUT[kt][:, tcol(ch)], AF.Identity,
                                     bias=vp[:, ccol + kt:ccol + kt + 1],
                                     scale=vp[:, gcol + kt:gcol + kt + 1])

        def ff_ch(ch, vp, W1d, W2d):
            wps = [ffpp.tile([128, 512], F32, tag=f"fa{dt}", name=f"fa{dt}")
                   for dt in range(4)]
            w1t = None
            for ff in range(16):
                ffb, fo = divmod(ff, 4)
                if fo == 0:
                    w1t = wpool.tile([128, 2048], F32, tag="w", name="w1t")
                    srcw = W1d[:].rearrange("p (kt f) -> p kt f", kt=4)
                    nc.sync.dma_start(
                        out=W_(w1t), in_=srcw[:, :, ffb * 512:(ffb + 1) * 512])
                    w2g = wpool.tile([128, 2048], F32, tag="w", name="w2g")
                    nc.sync.dma_start(
                        out=W_(w2g), in_=W2d[:, ffb * 2048:(ffb + 1) * 2048])
                ps1 = pp.tile([128, 512], F32, tag="mm")
                for kt in range(4):
                    nc.tensor.matmul(
                        ps1,
                        R(w1t[:, kt * 512 + fo * 128:kt * 512 + (fo + 1) * 128]),
                        R(Kb[kt][:, tcol(ch)]), start=kt == 0, stop=kt == 3)
                fr = frp.tile([128, 512], F32, tag="fr")
                nc.scalar.activation(W_(fr), ps1, AF.Relu,
                                     bias=vp[:, 28 + ff:29 + ff])
                for dt in range(4):
                    nc.tensor.matmul(
                        wps[dt],
                        R(w2g[:, fo * 512 + dt * 128:fo * 512 + (dt + 1) * 128]),
                        R(fr), start=ff == 0, stop=ff == 15)
            # u = ln1 + ffout + f2 -> VU (V is dead once attention finished)
            for dt in range(4):
                nc.vector.scalar_tensor_tensor(
                    out=W_(VU[dt][:, tcol(ch)]), in0=wps[dt],
                    scalar=vp[:, 24 + dt:25 + dt], in1=Kb[dt][:, tcol(ch)],
                    op0=ALU.add, op1=ALU.add)

        def xformer_layer(li):
            vp = vppool.tile([128, 44], F32, tag="vp")
            nc.sync.dma_start(out=vp, in_=din[f"L{li}_vp"][:])

            # Q = h@Wq + bq ; K = h@Wk (bk cancels in softmax)
            for dst, wname, bias in ((QA, "Wq", True), (Kb, "Wk", False)):
                wt = load_pk(din[f"L{li}_{wname}"], wname)
                for dt in range(4):
                    for ch in range(4):
                        ps = pp.tile([128, 512], F32, tag="mm")
                        for kt in range(4):
                            nc.tensor.matmul(
                                ps,
                                R(wt[:, kt * 512 + dt * 128:kt * 512 + (dt + 1) * 128]),
                                R(BUF[0][kt][:, tcol(ch)]),
                                start=kt == 0, stop=kt == 3)
                        if bias:
                            nc.vector.tensor_scalar_add(
                                W_(dst[dt][:, tcol(ch)]), ps, vp[:, dt:dt + 1])
                        else:
                            nc.vector.tensor_copy(W_(dst[dt][:, tcol(ch)]), ps)

            # V natural [tok, 8*(64+1)] with ones columns (denominator trick)
            for t in range(4):
                v3 = VU[t].rearrange("p (blk h x) -> p blk h x", blk=4, x=65)
                nc.sync.dma_start(out=W_(v3[:, :, :, 64:65]),
                                  in_=din["ones_v"][:].rearrange(
                                      "p (blk h) -> p blk h", blk=4))
            wt = load_pk(din[f"L{li}_Wv"], "Wv")
            for tt in range(16):
                ps = pp.tile([128, 512], F32, tag="mm")
                for kt in range(4):
                    nc.tensor.matmul(ps, R(BUF[0][kt][:, tt * 128:(tt + 1) * 128]),
                                     R(wt[:, kt * 512:(kt + 1) * 512]),
                                     start=kt == 0, stop=kt == 3)
                v3 = VU[tt // 4].rearrange("p (blk h x) -> p blk h x", blk=4, x=65)
                nc.scalar.activation(W_(v3[:, tt % 4, :, 0:64]),
                                     ps.rearrange("p (h x) -> p h x", x=64),
                                     AF.Copy)

            # attention per (batch, head) + per-chunk Wo/LN1/FF/LN2 slices
            wo = load_pk(din[f"L{li}_Wo"], "Wo")
            for b in range(BC):
                for hh in range(H):
                    tq, po = hh // 2, (hh % 2) * 64
                    qa = QA[tq][po:po + 64, bcol(b)]          # [64, 256]
                    pss = pp.tile([128, 512], F32, tag="mm")
                    nc.tensor.matmul(pss, R(eye_sb), R(mask_sb),
                                     start=True, stop=False)
                    for kc in range(2):
                        ka = Kb[tq][po:po + 64,
                                    b * 256 + kc * 128:b * 256 + kc * 128 + 128]
                        nc.tensor.matmul(pss[:, kc * 256:(kc + 1) * 256],
                                         R(ka), R(qa), start=False, stop=True)
                    et = probsp.tile([128, 512], F32, tag="probs")
                    nc.scalar.activation(W_(et), pss, AF.Exp, scale=0.125)
                    pso = ffpp.tile([65, 256], F32, tag=f"fa{hh % 4}",
                                    name="pso")
                    for kc in range(2):
                        tt = 2 * b + kc
                        va = VU[tt // 4][:, (tt % 4) * 520 + hh * 65:
                                         (tt % 4) * 520 + hh * 65 + 65]
                        nc.tensor.matmul(pso, R(va),
                                         R(et[:, kc * 256:(kc + 1) * 256]),
                                         start=kc == 0, stop=kc == 1)
                    # normalize + evict in one DVE op: A = oT * (1/denom)
                    dr0 = probsp.tile([1, 256], F32, tag="dr0", bufs=2)
                    nc.scalar.activation(dr0, pso[64:65, :], AF.Copy)
                    dr = probsp.tile([1, 256], F32, tag="dr", bufs=2)
                    nc.vector.reciprocal_approx_fast(dr, dr0)
                    bcr = probsp.tile([64, 256], F32, tag="bcr", bufs=2)
                    nc.gpsimd.partition_broadcast(bcr, dr)
                    nc.vector.tensor_mul(W_(QA[tq][po:po + 64, bcol(b)]),
                                         pso[0:64, :], bcr)
                if b % 2 == 1:
                    ch = b // 2
                    # t = h + A@Wo + bo_eff -> QA (A dead after these matmuls)
                    psl = [ffpp.tile([128, 512], F32, tag=f"fa{dt}",
                                     name=f"wo{dt}") for dt in range(4)]
                    for dt in range(4):
                        for kt in range(4):
                            nc.tensor.matmul(
                                psl[dt],
                                R(wo[:, kt * 512 + dt * 128:kt * 512 + (dt + 1) * 128]),
                                R(QA[kt][:, tcol(ch)]),
                                start=kt == 0, stop=kt == 3)
                    for dt in range(4):
                        nc.vector.scalar_tensor_tensor(
                            out=W_(QA[dt][:, tcol(ch)]), in0=psl[dt],
                            scalar=vp[:, 4 + dt:5 + dt],
                            in1=BUF[0][dt][:, tcol(ch)],
                            op0=ALU.add, op1=ALU.add)

            # phase-wise: LN1 (all chunks) -> FF -> LN2
            for ch in range(4):
                layernorm_ch(QA, VU, Kb, 8, 12, ch, vp)   # ln1 -> Kb
            for ch in range(4):
                ff_ch(ch, vp, din[f"L{li}_W1"], din[f"L{li}_W2"])  # u -> QA
            for ch in range(4):
                layernorm_ch(VU, QA, Hb, 16, 20, ch, vp)  # h_next -> Hb

        for li in range(NE):
            xformer_layer(li)

        # ---------------- latent head ----------------
        def pooled_ap(kt):
            # column b*256 of Hb[kt] for each b -> [128, 8, 1]
            return Hb[kt][:, 0:2048].rearrange("p (b s) -> p b s", s=256)[:, :, 0:1]

        lat_sb = {}
        for wname, gc, od in (("pm_w_pk", 0, mean_d), ("pl_w_pk", 1, lv_d)):
            w = wpool.tile([128, 128], F32, tag="w", name="latw")
            nc.sync.dma_start(out=W_(w), in_=din[wname][:])
            ps = pp.tile([LAT, BC], F32, tag="mm")
            for kt in range(4):
                nc.tensor.matmul(ps, R(w[:, kt * 32:(kt + 1) * 32]),
                                 R(pooled_ap(kt)), start=kt == 0, stop=kt == 3)
            res = smallp.tile([LAT, BC], F32, tag=f"lat{gc}")
            nc.scalar.activation(res, ps, AF.Identity, bias=gvp[0:LAT, gc:gc + 1])
            nc.sync.dma_start(out=od[:], in_=res)
            lat_sb[wname] = res

        ee = smallp.tile([LAT, BC], F32, tag="ee")
        nc.scalar.activation(ee, lat_sb["pl_w_pk"], AF.Exp, scale=0.5)
        ept = smallp.tile([LAT, BC], F32, tag="ept")
        nc.sync.dma_start(out=ept, in_=din["epsT"][:])
        zt = smallp.tile([LAT, BC], F32, tag="zt")
        nc.vector.tensor_mul(ee, ee, ept)
        nc.vector.tensor_add(W_(zt), lat_sb["pm_w_pk"], ee)

        # ---------------- expansion + conv -> decoder h0 ----------------
        ewt = wpool.tile([LAT, 512], F32, tag="w", name="ewt")
        nc.sync.dma_start(out=W_(ewt), in_=din["exp_w"][:])
        E_sb = smallp.tile([128, 32], F32, tag="E")
        for mt in range(4):
            ps = pp.tile([128, BC], F32, tag="mm")
            nc.tensor.matmul(ps, R(ewt[:, mt * 128:(mt + 1) * 128]), R(zt),
                             start=True, stop=True)
            nc.scalar.activation(W_(E_sb[:, mt * 8:(mt + 1) * 8]), ps, AF.Identity,
                                 bias=gvp[:, 2 + mt:3 + mt])
        for mt in range(4):
            nc.sync.dma_start(out=E_scr[mt * 128:(mt + 1) * 128, :],
                              in_=W_(E_sb[:, mt * 8:(mt + 1) * 8]))

        # S6[(i,t), (b,s)] = E[i, s+t-1, b] with zero padding at the edges
        s6 = initp.tile([17, TOK], F32, tag="init")
        z16 = din["zeros16"][:]
        for i in range(2):
            ez = s6[i * 3 + 0:i * 3 + 1].rearrange("p (b s) -> p b s", s=256)
            nc.sync.dma_start(out=W_(ez[:, :, 0:1]), in_=z16[:, 0:8])
            ez2 = s6[i * 3 + 2:i * 3 + 3].rearrange("p (b s) -> p b s", s=256)
            nc.sync.dma_start(out=W_(ez2[:, :, 255:256]), in_=z16[:, 0:8])
        for i in range(2):
            for t in range(3):
                d = t - 1
                cnt = 256 - abs(d)
                ds0, ss0 = max(0, -d), max(0, d)
                for b in range(BC):
                    r = i * 3 + t
                    dst = s6[r:r + 1, b * 256 + ds0:b * 256 + ds0 + cnt]
                    src = E_scr[i * 256 + ss0:i * 256 + ss0 + cnt, b:b + 1]
                    nc.sync.dma_start(out=W_(dst), in_=src)

        pos_t2 = wpool.tile([128, 1024], F32, tag="w", name="pos_t2")
        nc.sync.dma_start(out=pos_t2, in_=din["out_pos_pk"][:])
        w6t = wpool.tile([6, 512], F32, tag="w", name="w6t")
        nc.sync.dma_start(out=W_(w6t), in_=din["w6"][:])
        for mt in range(4):
            for ch in range(4):
                ps = pp.tile([128, 512], F32, tag="mm")
                nc.tensor.matmul(ps, R(w6t[:, mt * 128:(mt + 1) * 128]),
                                 R(s6[0:6, tcol(ch)]), start=True, stop=True)
                nc.vector.scalar_tensor_tensor(
                    out=W_(Hb[mt][:, tcol(ch)]), in0=ps,
                    scalar=gvp[:, 6 + mt:7 + mt], in1=posrep(pos_t2, mt),
                    op0=ALU.add, op1=ALU.add)

        for li in range(NE, NL):
            xformer_layer(li)

        # ---------------- output projection ----------------
        swt = wpool.tile([128, 64], F32, tag="w", name="swt")
        nc.sync.dma_start(out=W_(swt), in_=din["seq_wT_pk"][:])
        outsb = initp.tile([17, TOK], F32, tag="init", name="outsb")[0:BLK, :]
        for ch in range(4):
            ps = pp.tile([BLK, 512], F32, tag="mm")
            for kt in range(4):
                nc.tensor.matmul(ps, R(swt[:, kt * 16:(kt + 1) * 16]),
                                 R(Hb[kt][:, tcol(ch)]),
                                 start=kt == 0, stop=kt == 3)
            nc.scalar.activation(outsb[:, tcol(ch)], ps, AF.Identity,
                                 bias=gvp[0:BLK, 10:11])
        nc.sync.dma_start(out=outT_d[:], in_=outsb)

    nc.compile()
    return nc


_PROGRAM = None


def kernel(x, eps, params):
    global _PROGRAM
    from concourse.bass_utils import run_bass_kernel_spmd

    if _PROGRAM is None:
        _PROGRAM = _build_program()
    in_maps = _host_prep(x, eps, params)
    res = run_bass_kernel_spmd(_PROGRAM, in_maps, core_ids=list(range(NCORES)))

    decoded = np.zeros((BFULL, S * BLK), np.float32)
    means = np.zeros((BFULL, LAT), np.float32)
    lvs = np.zeros((BFULL, LAT), np.float32)
    for c in range(NCORES):
        r = res.results[c]
        # outT [16, 2048] -> decoded[b, s*16+o]
        decoded[c * BC:(c + 1) * BC] = (
            r["outT"].reshape(BLK, BC, S).transpose(1, 2, 0).reshape(BC, S * BLK))
        means[c * BC:(c + 1) * BC] = r["meanT"].T
        lvs[c * BC:(c + 1) * BC] = r["logvarT"].T
    elbo = np.float32(np.mean(
        -0.5 * np.sum(1.0 + lvs - means ** 2 - np.exp(lvs), axis=-1)))
    return decoded, elbo
